# revision 1
# baseline (speedup 1.0000x reference)
"""CrossViewTokenBlock Trainium2 kernel.

Problem: two token streams [B=4, T=1024, D=768]; each stream gets
LN -> cross-attention (12 heads, rel-pos bias) -> residual -> LN -> MLP -> residual,
with queries from its own stream and K/V from the other stream.

Sharding: the two streams' outputs are fully independent given the two
inputs, so the 8 cores each take one (stream, batch) pair:
cores 0-3 = stream 1 / batch 0-3, cores 4-7 = stream 2 / batch 0-3.
No collectives needed. A single SPMD Bass program runs on all 8 cores;
per-core differences (tokens, weights) come via the input maps.

Per-core math (all matmuls bf16 with fp32 PSUM accumulation):
 - LN gammas are folded into the following weight matrix (host, exact);
   LN betas become b @ W rows added as per-output biases.
 - Activations are kept feature-major ("transposed", [D, T]) for matmul
   chains; LN / softmax-normalize / residual run token-major.  PE
   transposes (via identity matmul) hop between the two layouts.
 - Attention computes logits^T [k, q] per (head, key-tile); the rel-pos
   bias (i-j)*wr_h reduces to a per-key -wr_h*j term (the +wr_h*i row
   term cancels in softmax normalization), applied as the per-partition
   bias of the exp() activation.  No max-subtraction (exponents are
   bounded ~|22| for this data distribution, safe in fp32).
 - The softmax denominator comes for free from a ones-column appended to
   V in the AV matmul; normalization is a per-partition scalar multiply
   straight out of PSUM.
"""

import numpy as np
import ml_dtypes

P = 128
T = 1024
D = 768
H = 12
HD = 64
HID = 3072
B = 4
EPS = 1e-6
NT = T // P      # 8 token tiles
ND = D // P      # 6 feature chunks
NH = HID // P    # 24 hidden chunks
SCALE = HD ** -0.5

BF = ml_dtypes.bfloat16


def build_nc(sim_gelu=False):
    import concourse.mybir as mybir
    import concourse.tile as tile
    from concourse import bacc
    from concourse.masks import make_identity

    f32 = mybir.dt.float32
    bf16 = mybir.dt.bfloat16
    AF = mybir.ActivationFunctionType
    OP = mybir.AluOpType

    # Bacc (not raw Bass): its compile() pipeline splits multi-sem waits into
    # event semaphores (TRN2 allows 1 wait/instruction) and inserts ACT table
    # loads -- raw Bass output fails walrus codegen with "Too many sync wait".
    nc = bacc.Bacc(None, target_bir_lowering=False)

    xs_d = nc.dram_tensor("xs", [T, D], f32, kind="ExternalInput")
    xo_d = nc.dram_tensor("xo", [T, D], f32, kind="ExternalInput")
    wq_d = nc.dram_tensor("wq", [D, D], bf16, kind="ExternalInput")
    wkv_d = nc.dram_tensor("wkv", [D, 2 * D], bf16, kind="ExternalInput")
    wp_d = nc.dram_tensor("wp", [D, D], bf16, kind="ExternalInput")
    w1_d = nc.dram_tensor("w1", [D, HID], bf16, kind="ExternalInput")
    w2_d = nc.dram_tensor("w2", [HID, D], bf16, kind="ExternalInput")
    qb_d = nc.dram_tensor("qb", [P, ND], f32, kind="ExternalInput")
    kb_d = nc.dram_tensor("kb", [P, ND], f32, kind="ExternalInput")
    bv_d = nc.dram_tensor("bv", [P, D], bf16, kind="ExternalInput")
    b1_d = nc.dram_tensor("b1", [P, NH], f32, kind="ExternalInput")
    b2_d = nc.dram_tensor("b2", [P, ND], f32, kind="ExternalInput")
    relb_d = nc.dram_tensor("relb", [P, H * NT], f32, kind="ExternalInput")
    out_d = nc.dram_tensor("out", [T, D], f32, kind="ExternalOutput")

    with tile.TileContext(nc) as tc:
        with (
            tc.tile_pool(name="persist", bufs=1) as persist,
            tc.tile_pool(name="stats", bufs=4) as stats,
            tc.tile_pool(name="norm", bufs=3) as norm_p,
            tc.tile_pool(name="actT", bufs=2) as actT_p,
            tc.tile_pool(name="outp", bufs=2) as out_p,
        ):
            ident = persist.tile([P, P], bf16)
            make_identity(nc, ident)
            eps_sb = persist.tile([P, 1], f32)
            nc.vector.memset(eps_sb[:], EPS)

            qb_sb = persist.tile([P, ND], f32)
            kb_sb = persist.tile([P, ND], f32)
            bv_sb = persist.tile([P, D], bf16)
            b1_sb = persist.tile([P, NH], f32)
            b2_sb = persist.tile([P, ND], f32)
            relb_sb = persist.tile([P, H * NT], f32)
            nc.sync.dma_start(qb_sb[:], qb_d[:])
            nc.sync.dma_start(kb_sb[:], kb_d[:])
            nc.sync.dma_start(bv_sb[:], bv_d[:])
            nc.sync.dma_start(b1_sb[:], b1_d[:])
            nc.sync.dma_start(b2_sb[:], b2_d[:])
            nc.sync.dma_start(relb_sb[:], relb_d[:])

            xs_sb = persist.tile([P, NT, D], f32)   # becomes r1 in place
            xs_t = xs_d[:].rearrange("(t p) d -> p t d", p=P)
            for t in range(NT):
                nc.sync.dma_start(xs_sb[:, t, :], xs_t[:, t, :])

            def layernorm_to(dst, src):
                """dst [P, D] bf16 <- normalized((src - mean) * rstd), token-major."""
                st = stats.tile([P, 3, 6], f32, tag="st")
                for s in range(3):
                    nc.vector.bn_stats(st[:, s, :], src[:, s * 256:(s + 1) * 256])
                mv = stats.tile([P, 2], f32, tag="mv")
                nc.vector.bn_aggr(mv[:], st[:])
                lnv = stats.tile([P, 1], f32, tag="lnv")
                rstd = stats.tile([P, 1], f32, tag="rstd")
                # rstd = exp(-0.5*ln(var+eps)): keeps ACT in the ln/exp table set
                nc.scalar.activation(lnv[:], mv[:, 1:2], AF.Ln, bias=eps_sb[:])
                nc.scalar.activation(rstd[:], lnv[:], AF.Exp, scale=-0.5)
                nc.vector.tensor_scalar(
                    dst, src, mv[:, 0:1], rstd[:],
                    op0=OP.subtract, op1=OP.mult,
                )

            def transpose_into(dstT, src, t, pool):
                """dstT [P, ND, T] bf16 <- transpose of token tile src [P, D] bf16."""
                for c in range(ND):
                    ps = pool.tile([P, P], bf16, tag="tr")
                    nc.tensor.transpose(ps[:], src[:, c * P:(c + 1) * P], ident[:])
                    nc.vector.tensor_copy(dstT[:, c, t * P:(t + 1) * P], ps[:])

            with (
                tc.tile_pool(name="attw", bufs=1) as attw,
                tc.tile_pool(name="attn", bufs=1) as attn_p,
            ):
                wq_sb = attw.tile([P, ND, D], bf16)
                wkv_sb = attw.tile([P, ND, 2 * D], bf16)
                wp_sb = attw.tile([P, ND, D], bf16)
                nc.sync.dma_start(wq_sb[:], wq_d[:].rearrange("(c p) n -> p c n", p=P))
                nc.sync.dma_start(wkv_sb[:], wkv_d[:].rearrange("(c p) n -> p c n", p=P))
                nc.sync.dma_start(wp_sb[:], wp_d[:].rearrange("(c p) n -> p c n", p=P))

                qT = attn_p.tile([P, ND, T], bf16)
                kT = attn_p.tile([P, ND, T], bf16)
                vt = attn_p.tile([P, NT, H, HD + 1], bf16)
                aout_n = attn_p.tile([P, NT, D], bf16)
                nc.gpsimd.memset(vt[:, :, :, HD:HD + 1], 1.0)

                # ---- Phase 1+2: LN_q / LN_kv, transpose to feature-major ----
                with (
                    tc.tile_pool(name="xop", bufs=1) as xo_p,
                    tc.tile_pool(name="psA_tr", bufs=2, space="PSUM") as psA_tr,
                    tc.tile_pool(name="psA_mm", bufs=3, space="PSUM") as psA_mm,
                ):
                    xo_sb = xo_p.tile([P, NT, D], f32)
                    xo_t = xo_d[:].rearrange("(t p) d -> p t d", p=P)
                    for t in range(NT):
                        nc.sync.dma_start(xo_sb[:, t, :], xo_t[:, t, :])

                    xqT = actT_p.tile([P, ND, T], bf16, tag="actT")
                    xkvT = actT_p.tile([P, ND, T], bf16, tag="actT")
                    for t in range(NT):
                        xq_n = norm_p.tile([P, D], bf16, tag="n")
                        layernorm_to(xq_n[:], xs_sb[:, t, :])
                        transpose_into(xqT, xq_n, t, psA_tr)
                    for t in range(NT):
                        xkv_n = norm_p.tile([P, D], bf16, tag="n")
                        layernorm_to(xkv_n[:], xo_sb[:, t, :])
                        transpose_into(xkvT, xkv_n, t, psA_tr)

                    # ---- Phase 3: Q^T, K^T (feature-major), V (token-major) ----
                    for m in range(ND):
                        for n2 in range(2):
                            ns = slice(n2 * 512, (n2 + 1) * 512)
                            ps = psA_mm.tile([P, 512], f32, tag="mm")
                            for c in range(ND):
                                nc.tensor.matmul(
                                    ps[:], wq_sb[:, c, m * P:(m + 1) * P],
                                    xqT[:, c, ns],
                                    start=(c == 0), stop=(c == ND - 1),
                                )
                            nc.vector.tensor_scalar_add(
                                qT[:, m, ns], ps[:], qb_sb[:, m:m + 1])
                    for m in range(ND):
                        for n2 in range(2):
                            ns = slice(n2 * 512, (n2 + 1) * 512)
                            ps = psA_mm.tile([P, 512], f32, tag="mm")
                            for c in range(ND):
                                nc.tensor.matmul(
                                    ps[:], wkv_sb[:, c, m * P:(m + 1) * P],
                                    xkvT[:, c, ns],
                                    start=(c == 0), stop=(c == ND - 1),
                                )
                            nc.vector.tensor_scalar_add(
                                kT[:, m, ns], ps[:], kb_sb[:, m:m + 1])
                    for kb in range(NT):
                        for off, nsz, h0, nh in ((0, 512, 0, 8), (512, 256, 8, 4)):
                            ps = psA_mm.tile([P, 512], f32, tag="mm")
                            for c in range(ND):
                                nc.tensor.matmul(
                                    ps[:, :nsz],
                                    xkvT[:, c, kb * P:(kb + 1) * P],
                                    wkv_sb[:, c, D + off:D + off + nsz],
                                    start=(c == 0), stop=(c == ND - 1),
                                )
                            nc.vector.tensor_copy(
                                vt[:, kb, h0:h0 + nh, 0:HD],
                                ps[:, :nsz].rearrange("p (h e) -> p h e", e=HD),
                            )

                # ---- Phase 4: attention ----
                with (
                    tc.tile_pool(name="pTp", bufs=2) as pT_p,
                    tc.tile_pool(name="psB_lg", bufs=2, space="PSUM") as psB_lg,
                    tc.tile_pool(name="psB_av", bufs=3, space="PSUM") as psB_av,
                ):
                    for h in range(H):
                        hs = slice((h % 2) * HD, (h % 2) * HD + HD)
                        hc = h // 2
                        pT = pT_p.tile([P, NT, T], bf16, tag="pT")
                        for kt in range(NT):
                            lg = psB_lg.tile([P, T], f32, tag="lg")
                            for n2 in range(2):
                                ns = slice(n2 * 512, (n2 + 1) * 512)
                                nc.tensor.matmul(
                                    lg[:, ns], kT[hs, hc, kt * P:(kt + 1) * P],
                                    qT[hs, hc, ns], start=True, stop=True,
                                )
                            ih = h * NT + kt
                            nc.scalar.activation(
                                pT[:, kt, :], lg[:], AF.Exp,
                                bias=relb_sb[:, ih:ih + 1], scale=SCALE,
                            )
                        for qb in range(NT):
                            av = psB_av.tile([P, HD + 1], f32, tag="av")
                            for kt in range(NT):
                                nc.tensor.matmul(
                                    av[:], pT[:, kt, qb * P:(qb + 1) * P],
                                    vt[:, kt, h, :],
                                    start=(kt == 0), stop=(kt == NT - 1),
                                )
                            rs = stats.tile([P, 1], f32, tag="rs")
                            nc.vector.reciprocal(rs[:], av[:, HD:HD + 1])
                            nc.vector.tensor_scalar_mul(
                                aout_n[:, qb, h * HD:(h + 1) * HD],
                                av[:, 0:HD], rs[:])

                # ---- Phase 5+6: +V-bias, transpose, out-proj, residual ----
                with (
                    tc.tile_pool(name="psC_tr", bufs=2, space="PSUM") as psC_tr,
                    tc.tile_pool(name="psC_mm", bufs=3, space="PSUM") as psC_mm,
                ):
                    for qb in range(NT):
                        nc.vector.tensor_tensor(
                            aout_n[:, qb, :], aout_n[:, qb, :], bv_sb[:], OP.add)
                    aoutT = actT_p.tile([P, ND, T], bf16, tag="actT")
                    for t in range(NT):
                        transpose_into(aoutT, aout_n[:, t, :], t, psC_tr)
                    for qb in range(NT):
                        for off, nsz in ((0, 512), (512, 256)):
                            ps = psC_mm.tile([P, 512], f32, tag="mm")
                            for c in range(ND):
                                nc.tensor.matmul(
                                    ps[:, :nsz],
                                    aoutT[:, c, qb * P:(qb + 1) * P],
                                    wp_sb[:, c, off:off + nsz],
                                    start=(c == 0), stop=(c == ND - 1),
                                )
                            nc.vector.tensor_add(
                                xs_sb[:, qb, off:off + nsz],
                                xs_sb[:, qb, off:off + nsz], ps[:, :nsz])

            # ---- Phase 7-10: LN_f, MLP, residual, store ----
            with (
                tc.tile_pool(name="mlpw", bufs=1) as mlpw,
                tc.tile_pool(name="gTp", bufs=1) as gT_p,
                tc.tile_pool(name="zTp", bufs=1) as zT_p,
                tc.tile_pool(name="psD_tr", bufs=2, space="PSUM") as psD_tr,
                tc.tile_pool(name="psD_mm", bufs=3, space="PSUM") as psD_mm,
            ):
                w1_sb = mlpw.tile([P, ND, HID], bf16)
                w2_sb = mlpw.tile([P, NH, D], bf16)
                nc.sync.dma_start(w1_sb[:], w1_d[:].rearrange("(c p) n -> p c n", p=P))
                nc.sync.dma_start(w2_sb[:], w2_d[:].rearrange("(c p) n -> p c n", p=P))

                mT = actT_p.tile([P, ND, T], bf16, tag="actT")
                for t in range(NT):
                    m_n = norm_p.tile([P, D], bf16, tag="n")
                    layernorm_to(m_n[:], xs_sb[:, t, :])
                    transpose_into(mT, m_n, t, psD_tr)

                zT = zT_p.tile([P, ND, T], bf16)
                for half in range(2):
                    ts_ = slice(half * 512, (half + 1) * 512)
                    gT = gT_p.tile([P, NH, 512], bf16, tag="gT")
                    for m in range(NH):
                        ps = psD_mm.tile([P, 512], f32, tag="mm")
                        for c in range(ND):
                            nc.tensor.matmul(
                                ps[:], w1_sb[:, c, m * P:(m + 1) * P],
                                mT[:, c, ts_],
                                start=(c == 0), stop=(c == ND - 1),
                            )
                        if not sim_gelu:
                            nc.scalar.activation(
                                gT[:, m, :], ps[:], AF.Gelu_apprx_tanh,
                                bias=b1_sb[:, m:m + 1])
                        else:
                            # CoreSim lacks Gelu: explicit tanh-approx gelu
                            cg, sg = 0.044715, 0.7978845608028654
                            xg = norm_p.tile([P, 512], f32, tag="xg")
                            nc.vector.tensor_scalar_add(
                                xg[:], ps[:], b1_sb[:, m:m + 1])
                            cu = norm_p.tile([P, 512], f32, tag="cu")
                            nc.scalar.activation(cu[:], xg[:], AF.Square)
                            nc.vector.tensor_tensor(
                                cu[:], cu[:], xg[:], OP.mult)
                            nc.vector.tensor_scalar(
                                cu[:], cu[:], float(sg * cg), None, op0=OP.mult)
                            ar = norm_p.tile([P, 512], f32, tag="ar")
                            nc.vector.tensor_scalar(
                                ar[:], xg[:], float(sg), None, op0=OP.mult)
                            nc.vector.tensor_tensor(ar[:], ar[:], cu[:], OP.add)
                            nc.scalar.activation(ar[:], ar[:], AF.Tanh)
                            nc.vector.tensor_scalar(
                                ar[:], ar[:], 1.0, 0.5,
                                op0=OP.add, op1=OP.mult)
                            nc.vector.tensor_tensor(
                                gT[:, m, :], ar[:], xg[:], OP.mult)
                    for m in range(ND):
                        ps = psD_mm.tile([P, 512], f32, tag="mm")
                        for c in range(NH):
                            nc.tensor.matmul(
                                ps[:], w2_sb[:, c, m * P:(m + 1) * P],
                                gT[:, c, :],
                                start=(c == 0), stop=(c == NH - 1),
                            )
                        nc.vector.tensor_scalar_add(
                            zT[:, m, ts_], ps[:], b2_sb[:, m:m + 1])

                for t in range(NT):
                    ob = out_p.tile([P, D], f32, tag="ob")
                    for c in range(ND):
                        ps = psD_tr.tile([P, P], bf16, tag="tr")
                        nc.tensor.transpose(
                            ps[:], zT[:, c, t * P:(t + 1) * P], ident[:])
                        nc.vector.tensor_add(
                            ob[:, c * P:(c + 1) * P],
                            xs_sb[:, t, c * P:(c + 1) * P], ps[:])
                    nc.sync.dma_start(out_d[t * P:(t + 1) * P, :], ob[:])

    # Bacc defers register allocation to finalize()/compile(); the
    # run_bass_kernel_spmd path serializes the module as-is, so run the
    # compile pipeline here (walrus rejects unallocated registers and
    # multi-sem waits otherwise).
    nc.finalize()
    return nc


def make_in_maps(inputs):
    """Host-side prep: fold LN gammas/betas into weights/biases, build the
    8 per-core input maps. cores 0-3: stream1 batch 0-3; 4-7: stream2."""
    inp = {k: np.asarray(v) for k, v in inputs.items()}
    f32 = np.float32

    def stream_tensors(tag, snum, gq, bq, gkv, bkv, gf, bf_):
        Wq, Wkv, Wp, Wr = (inp["Wq" + tag], inp["Wkv" + tag],
                           inp["Wp" + tag], inp["Wr" + tag])
        W1, b1v, W2, b2v = (inp["Wm" + snum + "a"], inp["bm" + snum + "a"],
                            inp["Wm" + snum + "b"], inp["bm" + snum + "b"])
        gq, bq, gkv, bkv, gf, bf_ = (inp[g].astype(f32) for g in
                                     (gq, bq, gkv, bkv, gf, bf_))
        qb = (bq @ Wq).astype(f32)
        kvb = (bkv @ Wkv).astype(f32)
        b1 = (b1v + bf_ @ W1).astype(f32)
        relb = np.empty((P, H * NT), f32)
        kk = np.arange(P, dtype=f32)
        for h in range(H):
            for kt in range(NT):
                relb[:, h * NT + kt] = -Wr[0, h] * (kt * P + kk)
        return {
            "wq": np.ascontiguousarray((gq[:, None] * Wq).astype(BF)),
            "wkv": np.ascontiguousarray((gkv[:, None] * Wkv).astype(BF)),
            "wp": np.ascontiguousarray(Wp.astype(BF)),
            "w1": np.ascontiguousarray((gf[:, None] * W1).astype(BF)),
            "w2": np.ascontiguousarray(W2.astype(BF)),
            "qb": np.ascontiguousarray(qb.reshape(ND, P).T),
            "kb": np.ascontiguousarray(kvb[:D].reshape(ND, P).T),
            "bv": np.ascontiguousarray(
                np.broadcast_to(kvb[D:].astype(BF), (P, D))),
            "b1": np.ascontiguousarray(b1.reshape(NH, P).T),
            "b2": np.ascontiguousarray(b2v.astype(f32).reshape(ND, P).T),
            "relb": relb,
        }

    s1 = stream_tensors("12", "1", "g_q1", "b_q1", "g_kv1", "b_kv1",
                        "g_f1", "b_f1")
    s2 = stream_tensors("21", "2", "g_q2", "b_q2", "g_kv2", "b_kv2",
                        "g_f2", "b_f2")
    t1 = np.ascontiguousarray(inp["tokens1"].astype(f32))
    t2 = np.ascontiguousarray(inp["tokens2"].astype(f32))

    in_maps = []
    for b in range(B):
        in_maps.append({"xs": t1[b], "xo": t2[b], **s1})
    for b in range(B):
        in_maps.append({"xs": t2[b], "xo": t1[b], **s2})
    return in_maps


_NC_CACHE = []


def kernel(**inputs):
    from concourse.bass_utils import run_bass_kernel_spmd

    if not _NC_CACHE:
        _NC_CACHE.append(build_nc())
    nc = _NC_CACHE[0]
    in_maps = make_in_maps(inputs)
    res = run_bass_kernel_spmd(nc, in_maps, core_ids=list(range(2 * B)))
    r = res.results
    tokens1 = np.stack([r[b]["out"] for b in range(B)]).astype(np.float32)
    tokens2 = np.stack([r[B + b]["out"] for b in range(B)]).astype(np.float32)
    return tokens1, tokens2



# revision 40
# speedup vs baseline: 18.2501x; 18.2501x over previous
"""CrossViewTokenBlock Trainium2 kernel.

Problem: two token streams [B=4, T=1024, D=768]; each stream gets
LN -> cross-attention (12 heads, rel-pos bias) -> residual -> LN -> MLP -> residual,
with queries from its own stream and K/V from the other stream.

Sharding: the two streams' outputs are fully independent given the two
inputs, so the 8 cores each take one (stream, batch) pair:
cores 0-3 = stream 1 / batch 0-3, cores 4-7 = stream 2 / batch 0-3.
No collectives needed. A single SPMD Bass program runs on all 8 cores;
per-core differences (tokens, weights) come via the input maps.

Per-core math (all matmuls bf16 with fp32 PSUM accumulation):
 - LN gammas are folded into the following weight matrix (host, exact);
   LN betas become b @ W rows added as per-output biases.
 - LN rstd = reciprocal(Sqrt(var+eps)): Sqrt on ACT, reciprocal on DVE.
   All Sqrt calls are emitted contiguously per phase so the ACT engine
   needs only ~4 activation-table loads for the whole kernel (Sqrt ->
   Exp -> Sqrt -> Gelu); the Ln/Exp-pair rstd of the old version caused
   48 table swaps at 1.3us each.
 - Activations are kept feature-major ("transposed", [D, T]) for matmul
   chains; LN / softmax-normalize / residual run token-major.  PE
   transposes (via identity matmul) hop between the two layouts; the 6
   chunk transposes of a token tile land in one PSUM bank and drain with
   a single strided DVE copy.
 - Q/K/V projection is software-pipelined into the head loop per
   head-pair (one 128-feature chunk feeds heads 2m,2m+1), so the PE
   keeps busy while ACT runs the softmax exps of earlier heads.
 - Attention computes logits^T [k, q] per (head, q-half, key-tile); the
   rel-pos bias (i-j)*wr_h reduces to a per-key -wr_h*j term (the
   +wr_h*i row term cancels in softmax normalization), applied as the
   per-partition bias of the exp() activation.  No max-subtraction
   (exponents are bounded ~|22| for this data distribution, safe fp32).
 - The softmax denominator comes for free from a ones-column appended to
   V in the AV matmul; normalization is a per-partition scalar multiply
   straight out of PSUM.
 - DMA issue order = arrival order (one shared transfer resource):
   xs tiles first, then wq/wkv, xo tiles (4-deep streaming buffer),
   attention biases, wp, w1 (prefetched into a persistent pool so fc1
   never waits), and w2 into the space the attention weights release.
"""

import numpy as np
import ml_dtypes

P = 128
T = 1024
D = 768
H = 12
HD = 64
HID = 3072
B = 4
EPS = 1e-6
NT = T // P      # 8 token tiles
ND = D // P      # 6 feature chunks
NH = HID // P    # 24 hidden chunks
SCALE = HD ** -0.5

BF = ml_dtypes.bfloat16


def build_nc(sim_gelu=False, use_gpsimd=True, use_ttr=False, use_sqacc=False):
    import concourse.mybir as mybir
    import concourse.tile as tile
    from concourse import bacc
    from concourse.masks import make_identity

    f32 = mybir.dt.float32
    bf16 = mybir.dt.bfloat16
    AF = mybir.ActivationFunctionType
    OP = mybir.AluOpType

    # Bacc (not raw Bass): its compile() pipeline splits multi-sem waits into
    # event semaphores (TRN2 allows 1 wait/instruction) and inserts ACT table
    # loads -- raw Bass output fails walrus codegen with "Too many sync wait".
    nc = bacc.Bacc(None, target_bir_lowering=False)

    xs_d = nc.dram_tensor("xs", [T, D], f32, kind="ExternalInput")
    xsb_d = nc.dram_tensor("xsb", [T, D], bf16, kind="ExternalInput")
    xob_d = nc.dram_tensor("xob", [T, D], bf16, kind="ExternalInput")
    wq_d = nc.dram_tensor("wq", [D, D], bf16, kind="ExternalInput")
    wk_d = nc.dram_tensor("wk", [D, D], bf16, kind="ExternalInput")
    wv_d = nc.dram_tensor("wv", [D, D], bf16, kind="ExternalInput")
    wp_d = nc.dram_tensor("wp", [D, D], bf16, kind="ExternalInput")
    w1_d = nc.dram_tensor("w1", [D, HID], bf16, kind="ExternalInput")
    w2_d = nc.dram_tensor("w2", [HID, D], bf16, kind="ExternalInput")
    qb_d = nc.dram_tensor("qb", [P, ND], f32, kind="ExternalInput")
    kb_d = nc.dram_tensor("kb", [P, ND], f32, kind="ExternalInput")
    bv_d = nc.dram_tensor("bv", [P, D], bf16, kind="ExternalInput")
    b1_d = nc.dram_tensor("b1", [P, NH], f32, kind="ExternalInput")
    b2_d = nc.dram_tensor("b2", [P, ND], f32, kind="ExternalInput")
    srel_d = nc.dram_tensor("srel", [P, H * NT], f32, kind="ExternalInput")
    out_d = nc.dram_tensor("out", [T, D], f32, kind="ExternalOutput")

    with tile.TileContext(nc) as tc:
        with (
            tc.tile_pool(name="persist", bufs=1) as persist,
            tc.tile_pool(name="stats", bufs=4) as stats,
            tc.tile_pool(name="norm", bufs=3) as norm_p,
            tc.tile_pool(name="actT", bufs=2) as actT_p,
            tc.tile_pool(name="xop", bufs=4) as xo_p,
            tc.tile_pool(name="outp", bufs=2) as out_p,
            tc.tile_pool(name="w1p", bufs=1) as w1_p,
            tc.tile_pool(name="ps_tr", bufs=2, space="PSUM") as ps_tr,
            tc.tile_pool(name="ps_mm", bufs=2, space="PSUM") as ps_mm,
        ):
            ident = persist.tile([P, P], bf16)
            make_identity(nc, ident)
            eps_sb = persist.tile([P, 1], f32)
            nc.vector.memset(eps_sb[:], EPS)

            # bf16 token copies first on the DMA queue (half the bytes of
            # f32): LN starts as soon as tile 0 lands.  The f32 xs (residual
            # base) arrives much later, first needed by the out-proj
            # residual add.
            xs_sb = persist.tile([P, NT, D], f32)   # becomes r1 in place
            xs_t = xs_d[:].rearrange("(t p) d -> p t d", p=P)
            xsb_t = xsb_d[:].rearrange("(t p) d -> p t d", p=P)
            xob_t = xob_d[:].rearrange("(t p) d -> p t d", p=P)

            def layernorm_to(dst, src):
                """dst [P, D] bf16 <- (src - mean) * rstd, token-major.
                Stats on DVE, tiny Sqrt on ACT, and the 768-wide apply on the
                otherwise-idle gpsimd engine — ACT stays free for softmax
                exps (its stream is in-order, so any LN work queued there
                would delay attention)."""
                st = stats.tile([P, 3, 6], f32, tag="st")
                for s in range(3):
                    nc.vector.bn_stats(st[:, s, :], src[:, s * 256:(s + 1) * 256])
                mv = stats.tile([P, 2], f32, tag="mv")
                nc.vector.bn_aggr(mv[:], st[:])
                std = stats.tile([P, 1], f32, tag="sd")
                nc.scalar.activation(std[:], mv[:, 1:2], AF.Sqrt, bias=eps_sb[:])
                rstd = stats.tile([P, 1], f32, tag="rstd")
                nc.vector.reciprocal(rstd[:], std[:])
                eng = nc.gpsimd if use_gpsimd else nc.vector
                eng.tensor_scalar(
                    dst, src, mv[:, 0:1], rstd[:],
                    op0=OP.subtract, op1=OP.mult,
                )

            def transpose_into(dstT, src, t, copy_act=False):
                """dstT[:, :, t*P:(t+1)*P] <- transpose of token tile src
                [P, D] bf16: 6 PE transposes into one PSUM bank, one strided
                copy out (DVE, or ACT Copy which is in every table set)."""
                ps = ps_tr.tile([P, D], bf16, tag="tr")
                for c in range(ND):
                    nc.tensor.transpose(
                        ps[:, c * P:(c + 1) * P], src[:, c * P:(c + 1) * P],
                        ident[:])
                dst = dstT[:, :, t * P:(t + 1) * P]
                srcv = ps[:].rearrange("p (c x) -> p c x", x=P)
                if copy_act:
                    nc.scalar.copy(dst, srcv)
                else:
                    nc.vector.tensor_copy(dst, srcv)

            with (
                tc.tile_pool(name="attw", bufs=1) as attw,
                tc.tile_pool(name="attn", bufs=1) as attn_p,
                tc.tile_pool(name="xsp", bufs=4) as xsb_p,
                tc.tile_pool(name="qkp", bufs=4) as qk_p,
                tc.tile_pool(name="vp", bufs=2) as v_p,
                tc.tile_pool(name="pTp", bufs=2) as pT_p,
                tc.tile_pool(name="ps_lg", bufs=2, space="PSUM") as ps_lg,
            ):
                # streaming bf16 token tiles (bufs=4): the DMA into tile 4+
                # waits on the LN that read tile t-4, which resolves well
                # before the later weight DMAs queue behind it.
                # kv tokens first: the kv-side LN -> K-proj chain gates the
                # first softmax, so it gets the DMA queue head.
                xob_tiles = []
                for t in range(NT):
                    xob_sb = xo_p.tile([P, D], bf16, tag="xo")
                    nc.sync.dma_start(xob_sb[:], xob_t[:, t, :])
                    xob_tiles.append(xob_sb)
                xsb_tiles = []
                for t in range(4):
                    xsb_sb = xsb_p.tile([P, D], bf16, tag="xq")
                    nc.sync.dma_start(xsb_sb[:], xsb_t[:, t, :])
                    xsb_tiles.append(xsb_sb)
                wk_sb = attw.tile([P, ND, D], bf16)
                wq_sb = attw.tile([P, ND, D], bf16)
                nc.sync.dma_start(wk_sb[:], wk_d[:].rearrange("(c p) n -> p c n", p=P))
                nc.sync.dma_start(wq_sb[:], wq_d[:].rearrange("(c p) n -> p c n", p=P))
                for t in range(4, NT):
                    xsb_sb = xsb_p.tile([P, D], bf16, tag="xq")
                    nc.sync.dma_start(xsb_sb[:], xsb_t[:, t, :])
                    xsb_tiles.append(xsb_sb)
                wv_sb = attw.tile([P, ND, D], bf16)
                nc.sync.dma_start(wv_sb[:], wv_d[:].rearrange("(c p) n -> p c n", p=P))
                qb_sb = persist.tile([P, ND], f32)
                kb_sb = persist.tile([P, ND], f32)
                srel_sb = persist.tile([P, H * NT], f32)
                nc.sync.dma_start(qb_sb[:], qb_d[:])
                nc.sync.dma_start(kb_sb[:], kb_d[:])
                nc.sync.dma_start(srel_sb[:], srel_d[:])
                bv_sb = persist.tile([P, D], bf16)
                nc.sync.dma_start(bv_sb[:], bv_d[:])
                wp_sb = attw.tile([P, ND, D], bf16)
                nc.sync.dma_start(wp_sb[:], wp_d[:].rearrange("(c p) n -> p c n", p=P))
                w1_sb = w1_p.tile([P, ND, HID], bf16)
                nc.sync.dma_start(w1_sb[:], w1_d[:].rearrange("(c p) n -> p c n", p=P))
                b1_sb = persist.tile([P, NH], f32)
                b2_sb = persist.tile([P, ND], f32)
                nc.sync.dma_start(b1_sb[:], b1_d[:])
                nc.sync.dma_start(b2_sb[:], b2_d[:])
                # f32 residual base: first consumer is the out-proj residual
                # add, long after this lands
                for t in range(NT):
                    nc.sync.dma_start(xs_sb[:, t, :], xs_t[:, t, :])

                # ---- LN + transpose to feature-major, kv tiles first.
                # q-tile PSUM drains go via ACT (Copy, table-free) to keep
                # DVE on the critical kv chain. ----
                xqT = actT_p.tile([P, ND, T], bf16, tag="actT")
                xkvT = actT_p.tile([P, ND, T], bf16, tag="actT")
                for t in range(NT):
                    xkv_n = norm_p.tile([P, D], bf16, tag="n")
                    layernorm_to(xkv_n[:], xob_tiles[t][:])
                    transpose_into(xkvT, xkv_n, t)
                for t in range(NT):
                    xq_n = norm_p.tile([P, D], bf16, tag="n")
                    layernorm_to(xq_n[:], xsb_tiles[t][:])
                    transpose_into(xqT, xq_n, t)

                aout_n = attn_p.tile([P, NT, D], bf16)

                # ---- QKV projection software-pipelined into the head loop.
                # PE executes its stream in order, so the independent Q/K/V
                # matmuls of chunk hp+1 are emitted in quarters BETWEEN the
                # logits and AV blocks of chunk hp's heads: they fill the
                # PE wait on ACT's softmax exps. ----
                def q_proj(hp, qTc):
                    cs = slice(hp * P, (hp + 1) * P)
                    for half in range(2):
                        ns = slice(half * 512, (half + 1) * 512)
                        ps = ps_mm.tile([P, 512], f32, tag="mm")
                        for c in range(ND):
                            nc.tensor.matmul(
                                ps[:], wq_sb[:, c, cs], xqT[:, c, ns],
                                start=(c == 0), stop=(c == ND - 1),
                            )
                        nc.vector.tensor_scalar_add(
                            qTc[:, ns], ps[:], qb_sb[:, hp:hp + 1])

                def k_proj(hp, kTc):
                    cs = slice(hp * P, (hp + 1) * P)
                    for half in range(2):
                        ns = slice(half * 512, (half + 1) * 512)
                        ps = ps_mm.tile([P, 512], f32, tag="mm")
                        for c in range(ND):
                            nc.tensor.matmul(
                                ps[:], wk_sb[:, c, cs], xkvT[:, c, ns],
                                start=(c == 0), stop=(c == ND - 1),
                            )
                        nc.vector.tensor_scalar_add(
                            kTc[:, ns], ps[:], kb_sb[:, hp:hp + 1])

                def v_proj(hp, vhp, kbs):
                    for kb in kbs:
                        ps = ps_mm.tile([P, 512], f32, tag="mm")
                        for c in range(ND):
                            nc.tensor.matmul(
                                ps[:, :P], xkvT[:, c, kb * P:(kb + 1) * P],
                                wv_sb[:, c, hp * P:(hp + 1) * P],
                                start=(c == 0), stop=(c == ND - 1),
                            )
                        nc.vector.tensor_copy(
                            vhp[:, kb, :, 0:HD],
                            ps[:, :P].rearrange("p (h e) -> p h e", e=HD),
                        )
                        # Fold the rel-pos key factor e^{-w_h j} into V and
                        # the ones column (idle gpsimd): softmax normalization
                        # divides it back out, so exp() needs no per-key bias
                        # and two key tiles share one full-width activation.
                        veng = nc.gpsimd if use_gpsimd else nc.vector
                        for hh in range(2):
                            ih = (2 * hp + hh) * NT + kb
                            veng.tensor_scalar_mul(
                                vhp[:, kb, hh, :], vhp[:, kb, hh, :],
                                srel_sb[:, ih:ih + 1])

                def new_chunk():
                    qTc = qk_p.tile([P, T], bf16, tag="q")
                    kTc = qk_p.tile([P, T], bf16, tag="k")
                    vhp = v_p.tile([P, NT, 2, HD + 1], bf16, tag="v")
                    nc.gpsimd.memset(vhp[:, :, :, HD:HD + 1], 1.0)
                    return qTc, kTc, vhp

                def do_head_half(h, hh, half, qTc, kTc, vhp, filler):
                    hs = slice(hh * HD, hh * HD + HD)
                    qs = slice(half * 512, (half + 1) * 512)
                    pT = pT_p.tile([P, NT, 512], bf16, tag="pT")
                    for kt in range(0, NT, 2):
                        lg = ps_lg.tile([P, 2, 512], f32, tag="lg")
                        nc.tensor.matmul(
                            lg[:, 0, :], kTc[hs, kt * P:(kt + 1) * P],
                            qTc[hs, qs], start=True, stop=True)
                        nc.tensor.matmul(
                            lg[:, 1, :], kTc[hs, (kt + 1) * P:(kt + 2) * P],
                            qTc[hs, qs], start=True, stop=True)
                        nc.scalar.activation(
                            pT[:, kt:kt + 2, :], lg[:], AF.Exp, scale=SCALE)
                    filler()
                    av4 = ps_mm.tile([P, 512], f32, tag="mm")
                    for qr in range(4):
                        vs = slice(qr * (HD + 1), (qr + 1) * (HD + 1))
                        for kt in range(NT):
                            nc.tensor.matmul(
                                av4[:, vs], pT[:, kt, qr * P:(qr + 1) * P],
                                vhp[:, kt, hh, :],
                                start=(kt == 0), stop=(kt == NT - 1),
                            )
                    for qr in range(4):
                        qi = half * 4 + qr
                        o = qr * (HD + 1)
                        rs = stats.tile([P, 1], f32, tag="rs")
                        nc.vector.reciprocal(rs[:], av4[:, o + HD:o + HD + 1])
                        nc.vector.tensor_scalar_mul(
                            aout_n[:, qi, h * HD:(h + 1) * HD],
                            av4[:, o:o + HD], rs[:])

                cur = new_chunk()
                k_proj(0, cur[1])
                q_proj(0, cur[0])
                v_proj(0, cur[2], range(NT))

                for hp in range(ND):        # head pair = feature chunk
                    qTc, kTc, vhp = cur
                    if hp + 1 < ND:
                        nxt = new_chunk()
                        fillers = [
                            lambda: q_proj(hp + 1, nxt[0]),
                            lambda: k_proj(hp + 1, nxt[1]),
                            lambda: v_proj(hp + 1, nxt[2], range(0, 4)),
                            lambda: v_proj(hp + 1, nxt[2], range(4, NT)),
                        ]
                    else:
                        nxt = None
                        fillers = [lambda: None] * 4
                    for hh in range(2):
                        for half in range(2):
                            do_head_half(2 * hp + hh, hh, half, qTc, kTc,
                                         vhp, fillers[2 * hh + half])
                    cur = nxt

                # ---- +V-bias, transpose, out-proj, residual, LN_f — all
                # per q tile, so LN_f/mT trail each residual immediately and
                # fc1 can start while late tiles still project ----
                aoutT = actT_p.tile([P, ND, T], bf16, tag="actT")
                mT = actT_p.tile([P, ND, T], bf16, tag="actT")
                sq_scr = attn_p.tile([P, D], bf16)
                m_ns = []
                for qi in range(NT):
                    # bf16 SBUF add on the otherwise-idle gpsimd engine
                    (nc.gpsimd if use_gpsimd else nc.vector).tensor_tensor(
                        aout_n[:, qi, :], aout_n[:, qi, :], bv_sb[:], OP.add)
                    transpose_into(aoutT, aout_n[:, qi, :], qi)
                    # residual add fused with the LN_f sum statistic; the
                    # square statistic comes from an ACT Square accumulate
                    # (Square is in every table set)
                    suma = stats.tile([P, 1], f32, tag="suma")
                    for i, (off, nsz) in enumerate(((0, 512), (512, 256))):
                        ps = ps_mm.tile([P, 512], f32, tag="mm")
                        for c in range(ND):
                            nc.tensor.matmul(
                                ps[:, :nsz],
                                aoutT[:, c, qi * P:(qi + 1) * P],
                                wp_sb[:, c, off:off + nsz],
                                start=(c == 0), stop=(c == ND - 1),
                            )
                        if use_ttr:
                            nc.vector.tensor_tensor_reduce(
                                xs_sb[:, qi, off:off + nsz],
                                xs_sb[:, qi, off:off + nsz], ps[:, :nsz],
                                1.0, (0.0 if i == 0 else suma[:]),
                                OP.add, OP.add, suma[:],
                            )
                        else:
                            nc.vector.tensor_add(
                                xs_sb[:, qi, off:off + nsz],
                                xs_sb[:, qi, off:off + nsz], ps[:, :nsz])
                    if use_ttr and use_sqacc:
                        ssq = stats.tile([P, 1], f32, tag="ssq")
                        nc.scalar.activation(
                            sq_scr[:], xs_sb[:, qi, :], AF.Square,
                            accum_out=ssq[:])
                        msq = stats.tile([P, 1], f32, tag="msq")
                        nc.vector.tensor_scalar(
                            msq[:], suma[:], suma[:], 1.0 / (D * D),
                            op0=OP.mult, op1=OP.mult)
                        var = stats.tile([P, 1], f32, tag="var")
                        nc.vector.tensor_scalar(
                            var[:], ssq[:], 1.0 / D, msq[:],
                            op0=OP.mult, op1=OP.subtract)
                        std = stats.tile([P, 1], f32, tag="sd")
                        nc.scalar.activation(
                            std[:], var[:], AF.Sqrt, bias=eps_sb[:])
                        rstd = stats.tile([P, 1], f32, tag="rstd")
                        nc.vector.reciprocal(rstd[:], std[:])
                        negp = stats.tile([P, 1], f32, tag="negp")
                        nc.vector.tensor_scalar(
                            negp[:], suma[:], rstd[:], -1.0 / D,
                            op0=OP.mult, op1=OP.mult)
                    else:
                        st = stats.tile([P, 3, 6], f32, tag="st")
                        for s in range(3):
                            nc.vector.bn_stats(
                                st[:, s, :],
                                xs_sb[:, qi, s * 256:(s + 1) * 256])
                        mv = stats.tile([P, 2], f32, tag="mv")
                        nc.vector.bn_aggr(mv[:], st[:])
                        std = stats.tile([P, 1], f32, tag="sd")
                        nc.scalar.activation(
                            std[:], mv[:, 1:2], AF.Sqrt, bias=eps_sb[:])
                        rstd = stats.tile([P, 1], f32, tag="rstd")
                        nc.vector.reciprocal(rstd[:], std[:])
                        negp = stats.tile([P, 1], f32, tag="negp")
                        nc.vector.tensor_scalar(
                            negp[:], mv[:, 0:1], rstd[:], -1.0,
                            op0=OP.mult, op1=OP.mult)
                    m_n = norm_p.tile([P, D], bf16, tag="n")
                    (nc.gpsimd if use_gpsimd else nc.vector).tensor_scalar(
                        m_n[:], xs_sb[:, qi, :], rstd[:], negp[:],
                        op0=OP.mult, op1=OP.add)
                    m_ns.append(m_n)
                    # transpose lags 2 tiles so PE never waits the LN chain
                    if qi >= 2:
                        transpose_into(mT, m_ns[qi - 2], qi - 2)
                for qi in (NT - 2, NT - 1):
                    transpose_into(mT, m_ns[qi], qi)

            # ---- MLP, residual, store ----
            with (
                tc.tile_pool(name="w2p", bufs=1) as w2_p,
                tc.tile_pool(name="gTp", bufs=1) as gT_p,
                tc.tile_pool(name="zTp", bufs=1) as zT_p,
            ):
                w2_sb = w2_p.tile([P, NH, D], bf16)
                nc.sync.dma_start(w2_sb[:], w2_d[:].rearrange("(c p) n -> p c n", p=P))

                zT = zT_p.tile([P, ND, T], bf16)
                for half in range(2):
                    ts_ = slice(half * 512, (half + 1) * 512)
                    gT = gT_p.tile([P, NH, 512], bf16, tag="gT")
                    for m in range(NH):
                        ps = ps_mm.tile([P, 512], f32, tag="mm")
                        for c in range(ND):
                            nc.tensor.matmul(
                                ps[:], w1_sb[:, c, m * P:(m + 1) * P],
                                mT[:, c, ts_],
                                start=(c == 0), stop=(c == ND - 1),
                            )
                        if not sim_gelu:
                            nc.scalar.activation(
                                gT[:, m, :], ps[:], AF.Gelu_apprx_tanh,
                                bias=b1_sb[:, m:m + 1])
                        else:
                            # CoreSim lacks Gelu: explicit tanh-approx gelu
                            cg, sg = 0.044715, 0.7978845608028654
                            xg = norm_p.tile([P, 512], f32, tag="xg", bufs=1)
                            nc.vector.tensor_scalar_add(
                                xg[:], ps[:], b1_sb[:, m:m + 1])
                            cu = norm_p.tile([P, 512], f32, tag="cu", bufs=1)
                            nc.scalar.activation(cu[:], xg[:], AF.Square)
                            nc.vector.tensor_tensor(
                                cu[:], cu[:], xg[:], OP.mult)
                            nc.vector.tensor_scalar(
                                cu[:], cu[:], float(sg * cg), None, op0=OP.mult)
                            ar = norm_p.tile([P, 512], f32, tag="ar", bufs=1)
                            nc.vector.tensor_scalar(
                                ar[:], xg[:], float(sg), None, op0=OP.mult)
                            nc.vector.tensor_tensor(ar[:], ar[:], cu[:], OP.add)
                            nc.scalar.activation(ar[:], ar[:], AF.Tanh)
                            nc.vector.tensor_scalar(
                                ar[:], ar[:], 1.0, 0.5,
                                op0=OP.add, op1=OP.mult)
                            nc.vector.tensor_tensor(
                                gT[:, m, :], ar[:], xg[:], OP.mult)
                    for m in range(ND):
                        ps = ps_mm.tile([P, 512], f32, tag="mm")
                        for c in range(NH):
                            nc.tensor.matmul(
                                ps[:], w2_sb[:, c, m * P:(m + 1) * P],
                                gT[:, c, :],
                                start=(c == 0), stop=(c == NH - 1),
                            )
                        nc.vector.tensor_scalar_add(
                            zT[:, m, ts_], ps[:], b2_sb[:, m:m + 1])
                    # store this half's token tiles while the other half
                    # computes
                    for t in range(half * 4, half * 4 + 4):
                        ps = ps_tr.tile([P, D], bf16, tag="tr")
                        for c in range(ND):
                            nc.tensor.transpose(
                                ps[:, c * P:(c + 1) * P],
                                zT[:, c, t * P:(t + 1) * P], ident[:])
                        ob = out_p.tile([P, D], f32, tag="ob")
                        nc.vector.tensor_add(ob[:], xs_sb[:, t, :], ps[:])
                        nc.sync.dma_start(out_d[t * P:(t + 1) * P, :], ob[:])

    # Bacc defers register allocation to finalize()/compile(); the
    # run_bass_kernel_spmd path serializes the module as-is, so run the
    # compile pipeline here (walrus rejects unallocated registers and
    # multi-sem waits otherwise).
    nc.finalize()
    return nc


def make_in_maps(inputs):
    """Host-side prep: fold LN gammas/betas into weights/biases, build the
    8 per-core input maps. cores 0-3: stream1 batch 0-3; 4-7: stream2."""
    inp = {k: np.asarray(v) for k, v in inputs.items()}
    f32 = np.float32

    def stream_tensors(tag, snum, gq, bq, gkv, bkv, gf, bf_):
        Wq, Wkv, Wp, Wr = (inp["Wq" + tag], inp["Wkv" + tag],
                           inp["Wp" + tag], inp["Wr" + tag])
        W1, b1v, W2, b2v = (inp["Wm" + snum + "a"], inp["bm" + snum + "a"],
                            inp["Wm" + snum + "b"], inp["bm" + snum + "b"])
        gq, bq, gkv, bkv, gf, bf_ = (inp[g].astype(f32) for g in
                                     (gq, bq, gkv, bkv, gf, bf_))
        qb = (bq @ Wq).astype(f32)
        kvb = (bkv @ Wkv).astype(f32)
        b1 = (b1v + bf_ @ W1).astype(f32)
        srel = np.empty((P, H * NT), f32)
        kk = np.arange(P, dtype=np.float64)
        for h in range(H):
            for kt in range(NT):
                srel[:, h * NT + kt] = np.exp(
                    -float(Wr[0, h]) * (kt * P + kk))
        wkv_f = (gkv[:, None] * Wkv).astype(BF)
        return {
            "wq": np.ascontiguousarray((gq[:, None] * Wq).astype(BF)),
            "wk": np.ascontiguousarray(wkv_f[:, :D]),
            "wv": np.ascontiguousarray(wkv_f[:, D:]),
            "wp": np.ascontiguousarray(Wp.astype(BF)),
            "w1": np.ascontiguousarray((gf[:, None] * W1).astype(BF)),
            "w2": np.ascontiguousarray(W2.astype(BF)),
            "qb": np.ascontiguousarray(qb.reshape(ND, P).T),
            "kb": np.ascontiguousarray(kvb[:D].reshape(ND, P).T),
            "bv": np.ascontiguousarray(
                np.broadcast_to(kvb[D:].astype(BF), (P, D))),
            "b1": np.ascontiguousarray(b1.reshape(NH, P).T),
            "b2": np.ascontiguousarray(b2v.astype(f32).reshape(ND, P).T),
            "srel": srel,
        }

    s1 = stream_tensors("12", "1", "g_q1", "b_q1", "g_kv1", "b_kv1",
                        "g_f1", "b_f1")
    s2 = stream_tensors("21", "2", "g_q2", "b_q2", "g_kv2", "b_kv2",
                        "g_f2", "b_f2")
    t1 = np.ascontiguousarray(inp["tokens1"].astype(f32))
    t2 = np.ascontiguousarray(inp["tokens2"].astype(f32))
    t1b = np.ascontiguousarray(t1.astype(BF))
    t2b = np.ascontiguousarray(t2.astype(BF))

    in_maps = []
    for b in range(B):
        in_maps.append({"xs": t1[b], "xsb": t1b[b], "xob": t2b[b], **s1})
    for b in range(B):
        in_maps.append({"xs": t2[b], "xsb": t2b[b], "xob": t1b[b], **s2})
    return in_maps


_NC_CACHE = []


def kernel(**inputs):
    from concourse.bass_utils import run_bass_kernel_spmd

    if not _NC_CACHE:
        _NC_CACHE.append(build_nc())
    nc = _NC_CACHE[0]
    in_maps = make_in_maps(inputs)
    res = run_bass_kernel_spmd(nc, in_maps, core_ids=list(range(2 * B)))
    r = res.results
    tokens1 = np.stack([r[b]["out"] for b in range(B)]).astype(np.float32)
    tokens2 = np.stack([r[B + b]["out"] for b in range(B)]).astype(np.float32)
    return tokens1, tokens2


# revision 45
# speedup vs baseline: 19.5473x; 1.0711x over previous
"""CrossViewTokenBlock Trainium2 kernel.

Problem: two token streams [B=4, T=1024, D=768]; each stream gets
LN -> cross-attention (12 heads, rel-pos bias) -> residual -> LN -> MLP -> residual,
with queries from its own stream and K/V from the other stream.

Sharding: the two streams' outputs are fully independent given the two
inputs, so the 8 cores each take one (stream, batch) pair:
cores 0-3 = stream 1 / batch 0-3, cores 4-7 = stream 2 / batch 0-3.
No collectives needed. A single SPMD Bass program runs on all 8 cores;
per-core differences (tokens, weights) come via the input maps.

Per-core math (all matmuls bf16 with fp32 PSUM accumulation):
 - LN gammas are folded into the following weight matrix (host, exact);
   LN betas become b @ W rows added as per-output biases.
 - LN rstd = reciprocal(Sqrt(var+eps)): Sqrt on ACT, reciprocal on DVE.
   All Sqrt calls are emitted contiguously per phase so the ACT engine
   needs only ~4 activation-table loads for the whole kernel (Sqrt ->
   Exp -> Sqrt -> Gelu); the Ln/Exp-pair rstd of the old version caused
   48 table swaps at 1.3us each.
 - Activations are kept feature-major ("transposed", [D, T]) for matmul
   chains; LN / softmax-normalize / residual run token-major.  PE
   transposes (via identity matmul) hop between the two layouts; the 6
   chunk transposes of a token tile land in one PSUM bank and drain with
   a single strided DVE copy.
 - Q/K/V projection is software-pipelined into the head loop per
   head-pair (one 128-feature chunk feeds heads 2m,2m+1), so the PE
   keeps busy while ACT runs the softmax exps of earlier heads.
 - Attention computes logits^T [k, q] per (head, q-half, key-tile); the
   rel-pos bias (i-j)*wr_h reduces to a per-key -wr_h*j term (the
   +wr_h*i row term cancels in softmax normalization), applied as the
   per-partition bias of the exp() activation.  No max-subtraction
   (exponents are bounded ~|22| for this data distribution, safe fp32).
 - The softmax denominator comes for free from a ones-column appended to
   V in the AV matmul; normalization is a per-partition scalar multiply
   straight out of PSUM.
 - DMA issue order = arrival order (one shared transfer resource):
   xs tiles first, then wq/wkv, xo tiles (4-deep streaming buffer),
   attention biases, wp, w1 (prefetched into a persistent pool so fc1
   never waits), and w2 into the space the attention weights release.
"""

import numpy as np
import ml_dtypes

P = 128
T = 1024
D = 768
H = 12
HD = 64
HID = 3072
B = 4
EPS = 1e-6
NT = T // P      # 8 token tiles
ND = D // P      # 6 feature chunks
NH = HID // P    # 24 hidden chunks
SCALE = HD ** -0.5

BF = ml_dtypes.bfloat16


def build_nc(sim_gelu=False, use_gpsimd=True, use_ttr=False, use_sqacc=False):
    import concourse.mybir as mybir
    import concourse.tile as tile
    from concourse import bacc
    from concourse.masks import make_identity

    f32 = mybir.dt.float32
    bf16 = mybir.dt.bfloat16
    AF = mybir.ActivationFunctionType
    OP = mybir.AluOpType

    # Bacc (not raw Bass): its compile() pipeline splits multi-sem waits into
    # event semaphores (TRN2 allows 1 wait/instruction) and inserts ACT table
    # loads -- raw Bass output fails walrus codegen with "Too many sync wait".
    nc = bacc.Bacc(None, target_bir_lowering=False)

    xs_d = nc.dram_tensor("xs", [T, D], f32, kind="ExternalInput")
    xsb_d = nc.dram_tensor("xsb", [T, D], bf16, kind="ExternalInput")
    xob_d = nc.dram_tensor("xob", [T, D], bf16, kind="ExternalInput")
    wq_d = nc.dram_tensor("wq", [ND, P, ND, P], bf16, kind="ExternalInput")
    wk_d = nc.dram_tensor("wk", [ND, P, ND, P], bf16, kind="ExternalInput")
    wv_d = nc.dram_tensor("wv", [ND, P, ND, P], bf16, kind="ExternalInput")
    wp_d = nc.dram_tensor("wp", [D, D], bf16, kind="ExternalInput")
    w1_d = nc.dram_tensor("w1", [D, HID], bf16, kind="ExternalInput")
    w2_d = nc.dram_tensor("w2", [HID, D], bf16, kind="ExternalInput")
    qb_d = nc.dram_tensor("qb", [P, ND], f32, kind="ExternalInput")
    kb_d = nc.dram_tensor("kb", [P, ND], f32, kind="ExternalInput")
    bv_d = nc.dram_tensor("bv", [P, D], bf16, kind="ExternalInput")
    b1_d = nc.dram_tensor("b1", [P, NH], f32, kind="ExternalInput")
    b2_d = nc.dram_tensor("b2", [P, ND], f32, kind="ExternalInput")
    srel_d = nc.dram_tensor("srel", [P, H * NT], f32, kind="ExternalInput")
    out_d = nc.dram_tensor("out", [T, D], f32, kind="ExternalOutput")

    with tile.TileContext(nc) as tc:
        with (
            tc.tile_pool(name="persist", bufs=1) as persist,
            tc.tile_pool(name="stats", bufs=4) as stats,
            tc.tile_pool(name="norm", bufs=3) as norm_p,
            tc.tile_pool(name="actT", bufs=2) as actT_p,
            tc.tile_pool(name="xop", bufs=4) as xo_p,
            tc.tile_pool(name="outp", bufs=2) as out_p,
            tc.tile_pool(name="w1p", bufs=1) as w1_p,
            tc.tile_pool(name="ps_tr", bufs=2, space="PSUM") as ps_tr,
            tc.tile_pool(name="ps_mm", bufs=2, space="PSUM") as ps_mm,
        ):
            ident = persist.tile([P, P], bf16)
            make_identity(nc, ident)
            eps_sb = persist.tile([P, 1], f32)
            nc.vector.memset(eps_sb[:], EPS)

            # bf16 token copies first on the DMA queue (half the bytes of
            # f32): LN starts as soon as tile 0 lands.  The f32 xs (residual
            # base) arrives much later, first needed by the out-proj
            # residual add.
            xs_sb = persist.tile([P, NT, D], f32)   # becomes r1 in place
            xs_t = xs_d[:].rearrange("(t p) d -> p t d", p=P)
            xsb_t = xsb_d[:].rearrange("(t p) d -> p t d", p=P)
            xob_t = xob_d[:].rearrange("(t p) d -> p t d", p=P)

            def layernorm_to(dst, src):
                """dst [P, D] bf16 <- (src - mean) * rstd, token-major.
                Stats on DVE, tiny Sqrt on ACT, and the 768-wide apply on the
                otherwise-idle gpsimd engine — ACT stays free for softmax
                exps (its stream is in-order, so any LN work queued there
                would delay attention)."""
                st = stats.tile([P, 2, 6], f32, tag="st")
                for s in range(2):
                    nc.vector.bn_stats(st[:, s, :], src[:, s * 384:(s + 1) * 384])
                mv = stats.tile([P, 2], f32, tag="mv")
                nc.vector.bn_aggr(mv[:], st[:])
                std = stats.tile([P, 1], f32, tag="sd")
                nc.scalar.activation(std[:], mv[:, 1:2], AF.Sqrt, bias=eps_sb[:])
                rstd = stats.tile([P, 1], f32, tag="rstd")
                nc.vector.reciprocal(rstd[:], std[:])
                eng = nc.gpsimd if use_gpsimd else nc.vector
                eng.tensor_scalar(
                    dst, src, mv[:, 0:1], rstd[:],
                    op0=OP.subtract, op1=OP.mult,
                )

            def transpose_into(dstT, src, t, copy_act=False):
                """dstT[:, :, t*P:(t+1)*P] <- transpose of token tile src
                [P, D] bf16: 6 PE transposes into one PSUM bank, one strided
                copy out (DVE, or ACT Copy which is in every table set)."""
                ps = ps_tr.tile([P, D], bf16, tag="tr")
                for c in range(ND):
                    nc.tensor.transpose(
                        ps[:, c * P:(c + 1) * P], src[:, c * P:(c + 1) * P],
                        ident[:])
                dst = dstT[:, :, t * P:(t + 1) * P]
                srcv = ps[:].rearrange("p (c x) -> p c x", x=P)
                if copy_act:
                    nc.scalar.copy(dst, srcv)
                else:
                    nc.vector.tensor_copy(dst, srcv)

            with (
                tc.tile_pool(name="attw", bufs=1) as attw,
                tc.tile_pool(name="attn", bufs=1) as attn_p,
                tc.tile_pool(name="xsp", bufs=8) as xsb_p,
                tc.tile_pool(name="qkp", bufs=4) as qk_p,
                tc.tile_pool(name="vp", bufs=2) as v_p,
                tc.tile_pool(name="pTp", bufs=2) as pT_p,
                tc.tile_pool(name="ps_lg", bufs=2, space="PSUM") as ps_lg,
            ):
                # streaming bf16 token tiles (bufs=4): the DMA into tile 4+
                # waits on the LN that read tile t-4, which resolves well
                # before the later weight DMAs queue behind it.
                # kv tokens first: the kv-side LN -> K-proj chain gates the
                # first softmax, so it gets the DMA queue head.
                xob_tiles = []
                for t in range(NT):
                    xob_sb = xo_p.tile([P, D], bf16, tag="xo")
                    nc.sync.dma_start(xob_sb[:], xob_t[:, t, :])
                    xob_tiles.append(xob_sb)
                xsb_tiles = []
                for t in range(4):
                    xsb_sb = xsb_p.tile([P, D], bf16, tag="xq")
                    nc.sync.dma_start(xsb_sb[:], xsb_t[:, t, :])
                    xsb_tiles.append(xsb_sb)
                # column-block-major QKV weights: head-pair 0's blocks land
                # within ~2us so the first logits don't wait on full matrices
                # [P, hp, c, n] so each head-pair block is one contiguous
                # 1536B-per-partition DMA
                wk_sb = attw.tile([P, ND, ND, P], bf16)
                wq_sb = attw.tile([P, ND, ND, P], bf16)
                wv_sb = attw.tile([P, ND, ND, P], bf16)

                def w_block(dst_sb, src_d, hp):
                    nc.sync.dma_start(dst_sb[:, hp, :, :], src_d[hp])

                for hp in range(2):
                    w_block(wk_sb, wk_d, hp)
                    w_block(wq_sb, wq_d, hp)
                    w_block(wv_sb, wv_d, hp)
                for t in range(4, NT):
                    xsb_sb = xsb_p.tile([P, D], bf16, tag="xq")
                    nc.sync.dma_start(xsb_sb[:], xsb_t[:, t, :])
                    xsb_tiles.append(xsb_sb)
                for hp in range(2, ND):
                    w_block(wk_sb, wk_d, hp)
                    w_block(wq_sb, wq_d, hp)
                    w_block(wv_sb, wv_d, hp)
                qb_sb = persist.tile([P, ND], f32)
                kb_sb = persist.tile([P, ND], f32)
                srel_sb = persist.tile([P, H * NT], f32)
                nc.sync.dma_start(qb_sb[:], qb_d[:])
                nc.sync.dma_start(kb_sb[:], kb_d[:])
                nc.sync.dma_start(srel_sb[:], srel_d[:])
                bv_sb = persist.tile([P, D], bf16)
                nc.sync.dma_start(bv_sb[:], bv_d[:])
                wp_sb = attw.tile([P, ND, D], bf16)
                nc.sync.dma_start(wp_sb[:], wp_d[:].rearrange("(c p) n -> p c n", p=P))
                w1_sb = w1_p.tile([P, ND, HID], bf16)
                nc.sync.dma_start(w1_sb[:], w1_d[:].rearrange("(c p) n -> p c n", p=P))
                b1_sb = persist.tile([P, NH], f32)
                b2_sb = persist.tile([P, ND], f32)
                nc.sync.dma_start(b1_sb[:], b1_d[:])
                nc.sync.dma_start(b2_sb[:], b2_d[:])
                # f32 residual base: first consumer is the out-proj residual
                # add, long after this lands
                for t in range(NT):
                    nc.sync.dma_start(xs_sb[:, t, :], xs_t[:, t, :])

                # ---- LN + transpose to feature-major, kv tiles first.
                # q-tile PSUM drains go via ACT (Copy, table-free) to keep
                # DVE on the critical kv chain. ----
                xqT = actT_p.tile([P, ND, T], bf16, tag="actT")
                xkvT = actT_p.tile([P, ND, T], bf16, tag="actT")
                for t in range(NT):
                    xkv_n = norm_p.tile([P, D], bf16, tag="n")
                    layernorm_to(xkv_n[:], xob_tiles[t][:])
                    transpose_into(xkvT, xkv_n, t)

                def q_ln(t):
                    xq_n = norm_p.tile([P, D], bf16, tag="n")
                    layernorm_to(xq_n[:], xsb_tiles[t][:])
                    transpose_into(xqT, xq_n, t)

                aout_n = attn_p.tile([P, NT, D], bf16)

                # ---- QKV projection software-pipelined into the head loop.
                # PE executes its stream in order, so the independent Q/K/V
                # matmuls of chunk hp+1 are emitted in quarters BETWEEN the
                # logits and AV blocks of chunk hp's heads: they fill the
                # PE wait on ACT's softmax exps. ----
                def q_proj(hp, qTc, halves=(0, 1)):
                    cs = slice(hp * P, (hp + 1) * P)
                    for half in halves:
                        ns = slice(half * 512, (half + 1) * 512)
                        ps = ps_mm.tile([P, 512], f32, tag="mm")
                        for c in range(ND):
                            nc.tensor.matmul(
                                ps[:], wq_sb[:, hp, c, :], xqT[:, c, ns],
                                start=(c == 0), stop=(c == ND - 1),
                            )
                        nc.vector.tensor_scalar_add(
                            qTc[:, ns], ps[:], qb_sb[:, hp:hp + 1])

                def k_proj(hp, kTc):
                    cs = slice(hp * P, (hp + 1) * P)
                    for half in range(2):
                        ns = slice(half * 512, (half + 1) * 512)
                        ps = ps_mm.tile([P, 512], f32, tag="mm")
                        for c in range(ND):
                            nc.tensor.matmul(
                                ps[:], wk_sb[:, hp, c, :], xkvT[:, c, ns],
                                start=(c == 0), stop=(c == ND - 1),
                            )
                        nc.vector.tensor_scalar_add(
                            kTc[:, ns], ps[:], kb_sb[:, hp:hp + 1])

                def v_proj(hp, vhp, kbs):
                    for kb in kbs:
                        ps = ps_mm.tile([P, 512], f32, tag="mm")
                        for c in range(ND):
                            nc.tensor.matmul(
                                ps[:, :P], xkvT[:, c, kb * P:(kb + 1) * P],
                                wv_sb[:, hp, c, :],
                                start=(c == 0), stop=(c == ND - 1),
                            )
                        nc.vector.tensor_copy(
                            vhp[:, kb, :, 0:HD],
                            ps[:, :P].rearrange("p (h e) -> p h e", e=HD),
                        )
                        # Fold the rel-pos key factor e^{-w_h j} into V and
                        # the ones column (idle gpsimd): softmax normalization
                        # divides it back out, so exp() needs no per-key bias
                        # and two key tiles share one full-width activation.
                        veng = nc.gpsimd if use_gpsimd else nc.vector
                        for hh in range(2):
                            ih = (2 * hp + hh) * NT + kb
                            veng.tensor_scalar_mul(
                                vhp[:, kb, hh, :], vhp[:, kb, hh, :],
                                srel_sb[:, ih:ih + 1])

                def new_chunk():
                    qTc = qk_p.tile([P, T], bf16, tag="q")
                    kTc = qk_p.tile([P, T], bf16, tag="k")
                    vhp = v_p.tile([P, NT, 2, HD + 1], bf16, tag="v")
                    nc.gpsimd.memset(vhp[:, :, :, HD:HD + 1], 1.0)
                    return qTc, kTc, vhp

                def do_head_half(h, hh, half, qTc, kTc, vhp, filler):
                    hs = slice(hh * HD, hh * HD + HD)
                    qs = slice(half * 512, (half + 1) * 512)
                    pT = pT_p.tile([P, NT, 512], bf16, tag="pT")
                    for kt in range(0, NT, 2):
                        lg = ps_lg.tile([P, 2, 512], f32, tag="lg")
                        nc.tensor.matmul(
                            lg[:, 0, :], kTc[hs, kt * P:(kt + 1) * P],
                            qTc[hs, qs], start=True, stop=True)
                        nc.tensor.matmul(
                            lg[:, 1, :], kTc[hs, (kt + 1) * P:(kt + 2) * P],
                            qTc[hs, qs], start=True, stop=True)
                        nc.scalar.activation(
                            pT[:, kt:kt + 2, :], lg[:], AF.Exp, scale=SCALE)
                    filler()
                    av4 = ps_mm.tile([P, 512], f32, tag="mm")
                    for qr in range(4):
                        vs = slice(qr * (HD + 1), (qr + 1) * (HD + 1))
                        for kt in range(NT):
                            nc.tensor.matmul(
                                av4[:, vs], pT[:, kt, qr * P:(qr + 1) * P],
                                vhp[:, kt, hh, :],
                                start=(kt == 0), stop=(kt == NT - 1),
                            )
                    for qr in range(4):
                        qi = half * 4 + qr
                        o = qr * (HD + 1)
                        rs = stats.tile([P, 1], f32, tag="rs")
                        nc.vector.reciprocal(rs[:], av4[:, o + HD:o + HD + 1])
                        nc.vector.tensor_scalar_mul(
                            aout_n[:, qi, h * HD:(h + 1) * HD],
                            av4[:, o:o + HD], rs[:])

                # prologue chunk 0: K/V matmuls interleaved between the
                # q-tile LN transposes so PE isn't in-order-blocked on the
                # later q tiles
                cur = new_chunk()
                q_ln(0); q_ln(1)
                k_proj(0, cur[1])
                q_ln(2); q_ln(3)
                q_proj(0, cur[0], halves=(0,))
                v_proj(0, cur[2], range(0, 4))
                q_ln(4); q_ln(5)
                v_proj(0, cur[2], range(4, 6))
                q_ln(6); q_ln(7)
                q_proj(0, cur[0], halves=(1,))
                v_proj(0, cur[2], range(6, NT))

                for hp in range(ND):        # head pair = feature chunk
                    qTc, kTc, vhp = cur
                    if hp + 1 < ND:
                        nxt = new_chunk()
                        fillers = [
                            lambda: q_proj(hp + 1, nxt[0]),
                            lambda: k_proj(hp + 1, nxt[1]),
                            lambda: v_proj(hp + 1, nxt[2], range(0, 4)),
                            lambda: v_proj(hp + 1, nxt[2], range(4, NT)),
                        ]
                    else:
                        nxt = None
                        fillers = [lambda: None] * 4
                    for hh in range(2):
                        for half in range(2):
                            do_head_half(2 * hp + hh, hh, half, qTc, kTc,
                                         vhp, fillers[2 * hh + half])
                    cur = nxt

                # ---- +V-bias, transpose, out-proj, residual, LN_f — all
                # per q tile, so LN_f/mT trail each residual immediately and
                # fc1 can start while late tiles still project ----
                aoutT = actT_p.tile([P, ND, T], bf16, tag="actT")
                mT = actT_p.tile([P, ND, T], bf16, tag="actT")
                sq_scr = attn_p.tile([P, D], bf16)
                m_ns = []
                for qi in range(NT):
                    # bf16 SBUF add on the otherwise-idle gpsimd engine
                    (nc.gpsimd if use_gpsimd else nc.vector).tensor_tensor(
                        aout_n[:, qi, :], aout_n[:, qi, :], bv_sb[:], OP.add)
                    transpose_into(aoutT, aout_n[:, qi, :], qi)
                    # residual add; LN_f statistics either via ACT
                    # accumulators (Identity/Square are in every table set)
                    # or classic DVE bn_stats
                    suma = stats.tile([P, 1], f32, tag="suma")
                    for i, (off, nsz) in enumerate(((0, 512), (512, 256))):
                        ps = ps_mm.tile([P, 512], f32, tag="mm")
                        for c in range(ND):
                            nc.tensor.matmul(
                                ps[:, :nsz],
                                aoutT[:, c, qi * P:(qi + 1) * P],
                                wp_sb[:, c, off:off + nsz],
                                start=(c == 0), stop=(c == ND - 1),
                            )
                        if use_ttr:
                            nc.vector.tensor_tensor_reduce(
                                xs_sb[:, qi, off:off + nsz],
                                xs_sb[:, qi, off:off + nsz], ps[:, :nsz],
                                1.0, (0.0 if i == 0 else suma[:]),
                                OP.add, OP.add, suma[:],
                            )
                        else:
                            nc.vector.tensor_add(
                                xs_sb[:, qi, off:off + nsz],
                                xs_sb[:, qi, off:off + nsz], ps[:, :nsz])
                    if use_sqacc:
                        nc.scalar.activation(
                            sq_scr[:], xs_sb[:, qi, :], AF.Identity,
                            accum_out=suma[:])
                        ssq = stats.tile([P, 1], f32, tag="ssq")
                        nc.scalar.activation(
                            sq_scr[:], xs_sb[:, qi, :], AF.Square,
                            accum_out=ssq[:])
                        msq = stats.tile([P, 1], f32, tag="msq")
                        nc.vector.tensor_scalar(
                            msq[:], suma[:], suma[:], 1.0 / (D * D),
                            op0=OP.mult, op1=OP.mult)
                        var = stats.tile([P, 1], f32, tag="var")
                        nc.vector.tensor_scalar(
                            var[:], ssq[:], 1.0 / D, msq[:],
                            op0=OP.mult, op1=OP.subtract)
                        std = stats.tile([P, 1], f32, tag="sd")
                        nc.scalar.activation(
                            std[:], var[:], AF.Sqrt, bias=eps_sb[:])
                        rstd = stats.tile([P, 1], f32, tag="rstd")
                        nc.vector.reciprocal(rstd[:], std[:])
                        negp = stats.tile([P, 1], f32, tag="negp")
                        nc.vector.tensor_scalar(
                            negp[:], suma[:], rstd[:], -1.0 / D,
                            op0=OP.mult, op1=OP.mult)
                    else:
                        st = stats.tile([P, 2, 6], f32, tag="st")
                        for s in range(2):
                            nc.vector.bn_stats(
                                st[:, s, :],
                                xs_sb[:, qi, s * 384:(s + 1) * 384])
                        mv = stats.tile([P, 2], f32, tag="mv")
                        nc.vector.bn_aggr(mv[:], st[:])
                        std = stats.tile([P, 1], f32, tag="sd")
                        nc.scalar.activation(
                            std[:], mv[:, 1:2], AF.Sqrt, bias=eps_sb[:])
                        rstd = stats.tile([P, 1], f32, tag="rstd")
                        nc.vector.reciprocal(rstd[:], std[:])
                        negp = stats.tile([P, 1], f32, tag="negp")
                        nc.vector.tensor_scalar(
                            negp[:], mv[:, 0:1], rstd[:], -1.0,
                            op0=OP.mult, op1=OP.mult)
                    m_n = norm_p.tile([P, D], bf16, tag="n")
                    (nc.gpsimd if use_gpsimd else nc.vector).tensor_scalar(
                        m_n[:], xs_sb[:, qi, :], rstd[:], negp[:],
                        op0=OP.mult, op1=OP.add)
                    m_ns.append(m_n)
                    # transpose lags 2 tiles so PE never waits the LN chain
                    if qi >= 2:
                        transpose_into(mT, m_ns[qi - 2], qi - 2)
                for qi in (NT - 2, NT - 1):
                    transpose_into(mT, m_ns[qi], qi)

            # ---- MLP, residual, store ----
            with (
                tc.tile_pool(name="w2p", bufs=1) as w2_p,
                tc.tile_pool(name="gTp", bufs=1) as gT_p,
                tc.tile_pool(name="zTp", bufs=1) as zT_p,
            ):
                w2_sb = w2_p.tile([P, NH, D], bf16)
                nc.sync.dma_start(w2_sb[:], w2_d[:].rearrange("(c p) n -> p c n", p=P))

                zT = zT_p.tile([P, ND, T], bf16)
                for half in range(2):
                    ts_ = slice(half * 512, (half + 1) * 512)
                    gT = gT_p.tile([P, NH, 512], bf16, tag="gT")
                    for m in range(NH):
                        ps = ps_mm.tile([P, 512], f32, tag="mm")
                        for c in range(ND):
                            nc.tensor.matmul(
                                ps[:], w1_sb[:, c, m * P:(m + 1) * P],
                                mT[:, c, ts_],
                                start=(c == 0), stop=(c == ND - 1),
                            )
                        if not sim_gelu:
                            nc.scalar.activation(
                                gT[:, m, :], ps[:], AF.Gelu_apprx_tanh,
                                bias=b1_sb[:, m:m + 1])
                        else:
                            # CoreSim lacks Gelu: explicit tanh-approx gelu
                            cg, sg = 0.044715, 0.7978845608028654
                            xg = norm_p.tile([P, 512], f32, tag="xg", bufs=1)
                            nc.vector.tensor_scalar_add(
                                xg[:], ps[:], b1_sb[:, m:m + 1])
                            cu = norm_p.tile([P, 512], f32, tag="cu", bufs=1)
                            nc.scalar.activation(cu[:], xg[:], AF.Square)
                            nc.vector.tensor_tensor(
                                cu[:], cu[:], xg[:], OP.mult)
                            nc.vector.tensor_scalar(
                                cu[:], cu[:], float(sg * cg), None, op0=OP.mult)
                            ar = norm_p.tile([P, 512], f32, tag="ar", bufs=1)
                            nc.vector.tensor_scalar(
                                ar[:], xg[:], float(sg), None, op0=OP.mult)
                            nc.vector.tensor_tensor(ar[:], ar[:], cu[:], OP.add)
                            nc.scalar.activation(ar[:], ar[:], AF.Tanh)
                            nc.vector.tensor_scalar(
                                ar[:], ar[:], 1.0, 0.5,
                                op0=OP.add, op1=OP.mult)
                            nc.vector.tensor_tensor(
                                gT[:, m, :], ar[:], xg[:], OP.mult)
                    for m in range(ND):
                        ps = ps_mm.tile([P, 512], f32, tag="mm")
                        for c in range(NH):
                            nc.tensor.matmul(
                                ps[:], w2_sb[:, c, m * P:(m + 1) * P],
                                gT[:, c, :],
                                start=(c == 0), stop=(c == NH - 1),
                            )
                        nc.vector.tensor_scalar_add(
                            zT[:, m, ts_], ps[:], b2_sb[:, m:m + 1])
                    # store this half's token tiles while the other half
                    # computes
                    for t in range(half * 4, half * 4 + 4):
                        ps = ps_tr.tile([P, D], bf16, tag="tr")
                        for c in range(ND):
                            nc.tensor.transpose(
                                ps[:, c * P:(c + 1) * P],
                                zT[:, c, t * P:(t + 1) * P], ident[:])
                        ob = out_p.tile([P, D], f32, tag="ob")
                        nc.vector.tensor_add(ob[:], xs_sb[:, t, :], ps[:])
                        nc.sync.dma_start(out_d[t * P:(t + 1) * P, :], ob[:])

    # Bacc defers register allocation to finalize()/compile(); the
    # run_bass_kernel_spmd path serializes the module as-is, so run the
    # compile pipeline here (walrus rejects unallocated registers and
    # multi-sem waits otherwise).
    nc.finalize()
    return nc


def make_in_maps(inputs):
    """Host-side prep: fold LN gammas/betas into weights/biases, build the
    8 per-core input maps. cores 0-3: stream1 batch 0-3; 4-7: stream2."""
    inp = {k: np.asarray(v) for k, v in inputs.items()}
    f32 = np.float32

    def stream_tensors(tag, snum, gq, bq, gkv, bkv, gf, bf_):
        Wq, Wkv, Wp, Wr = (inp["Wq" + tag], inp["Wkv" + tag],
                           inp["Wp" + tag], inp["Wr" + tag])
        W1, b1v, W2, b2v = (inp["Wm" + snum + "a"], inp["bm" + snum + "a"],
                            inp["Wm" + snum + "b"], inp["bm" + snum + "b"])
        gq, bq, gkv, bkv, gf, bf_ = (inp[g].astype(f32) for g in
                                     (gq, bq, gkv, bkv, gf, bf_))
        qb = (bq @ Wq).astype(f32)
        kvb = (bkv @ Wkv).astype(f32)
        b1 = (b1v + bf_ @ W1).astype(f32)
        srel = np.empty((P, H * NT), f32)
        kk = np.arange(P, dtype=np.float64)
        for h in range(H):
            for kt in range(NT):
                srel[:, h * NT + kt] = np.exp(
                    -float(Wr[0, h]) * (kt * P + kk))
        wkv_f = (gkv[:, None] * Wkv).astype(BF)

        def colblocks(w):
            # [hp, p, c, n]: block hp = w[:, hp*128:(hp+1)*128] with its
            # 768 rows regrouped (c, p) -> partition-major
            return np.ascontiguousarray(
                w.reshape(ND, P, ND, P).transpose(2, 1, 0, 3))

        return {
            "wq": colblocks((gq[:, None] * Wq).astype(BF)),
            "wk": colblocks(wkv_f[:, :D]),
            "wv": colblocks(wkv_f[:, D:]),
            "wp": np.ascontiguousarray(Wp.astype(BF)),
            "w1": np.ascontiguousarray((gf[:, None] * W1).astype(BF)),
            "w2": np.ascontiguousarray(W2.astype(BF)),
            "qb": np.ascontiguousarray(qb.reshape(ND, P).T),
            "kb": np.ascontiguousarray(kvb[:D].reshape(ND, P).T),
            "bv": np.ascontiguousarray(
                np.broadcast_to(kvb[D:].astype(BF), (P, D))),
            "b1": np.ascontiguousarray(b1.reshape(NH, P).T),
            "b2": np.ascontiguousarray(b2v.astype(f32).reshape(ND, P).T),
            "srel": srel,
        }

    s1 = stream_tensors("12", "1", "g_q1", "b_q1", "g_kv1", "b_kv1",
                        "g_f1", "b_f1")
    s2 = stream_tensors("21", "2", "g_q2", "b_q2", "g_kv2", "b_kv2",
                        "g_f2", "b_f2")
    t1 = np.ascontiguousarray(inp["tokens1"].astype(f32))
    t2 = np.ascontiguousarray(inp["tokens2"].astype(f32))
    t1b = np.ascontiguousarray(t1.astype(BF))
    t2b = np.ascontiguousarray(t2.astype(BF))

    in_maps = []
    for b in range(B):
        in_maps.append({"xs": t1[b], "xsb": t1b[b], "xob": t2b[b], **s1})
    for b in range(B):
        in_maps.append({"xs": t2[b], "xsb": t2b[b], "xob": t1b[b], **s2})
    return in_maps


_NC_CACHE = []


def kernel(**inputs):
    from concourse.bass_utils import run_bass_kernel_spmd

    if not _NC_CACHE:
        _NC_CACHE.append(build_nc())
    nc = _NC_CACHE[0]
    in_maps = make_in_maps(inputs)
    res = run_bass_kernel_spmd(nc, in_maps, core_ids=list(range(2 * B)))
    r = res.results
    tokens1 = np.stack([r[b]["out"] for b in range(B)]).astype(np.float32)
    tokens2 = np.stack([r[B + b]["out"] for b in range(B)]).astype(np.float32)
    return tokens1, tokens2
